# revision 1
# baseline (speedup 1.0000x reference)
"""AFNO spectral attention kernel for 8 TRN2 NeuronCores.

Math: the reference's rfft2 -> truncate -> per-block mode mix -> irfft2
collapses to a per-block real 224x224 matrix A_b applied along the W axis
(the H-direction FFT commutes with the mode mixing and cancels), plus a
bias-driven constant on the n_h==0 spatial rows. The final projection
folds into P = rescale*proj_w.T + I so the device only runs two matmul
stages:
  stage 1: Xs[r, w, c] = sum_w' X[r, w', c] * A_{b(c)}[w', w]
  stage 2: out[t, o]   = sum_c Xs[t, c] * P[c, o]  (+ rescale*proj_b)

Sharding: 100352 tokens = 8 cores x 12544 (56 complete image rows per
core, batch boundary lands exactly on the core-4 boundary). No
collectives needed.

Layout notes: stage-1 runs one matmul set per 96-wide channel block;
each block's PSUM is then repacked into six channel-contiguous K=128
contraction chunks for stage-2 -- the two chunk-aligned blocks leave
via plain ACT copies, the rest via 32-wide partition-shifted DVE
copies (the DVE's 32-lane bank can write any quadrant; wider copies
cannot shift across quadrants). Six K=128 chunks instead of the
padded 3x128+4x96 split keeps stage-2 at the PE streaming roofline
(measured 216+109 ns per 512+256 output pair, back to back). Output
is written bf16 (error budget allows) and upcast on the host, halving
store traffic.
"""

import numpy as np
import ml_dtypes

import concourse.bass as bass
import concourse.mybir as mybir
import concourse.tile as tile
from concourse.bass_utils import run_bass_kernel_spmd

B, Hh, Ww, C = 2, 224, 224, 768
NB, BS, M = 8, 96, 96
NMODES = Ww // 2 + 1  # 113
N_CORES = 8
TOK = B * Hh * Ww  # 100352 total tokens
TOK_CORE = TOK // N_CORES  # 12544
ROWS_CORE = TOK_CORE // Ww  # 56 image rows per core
RG = 4  # image rows per group
GROUPS = ROWS_CORE // RG  # 14
TG = RG * Ww  # tokens per group = 896
TCH = TG // 128  # t-chunks of 128 per group = 7
NS = 6  # stage-2 contraction chunks (128 channels each)
A_COLS = 2 * NB * Ww  # 3584
P_COLS = NS * C  # 4608
WC_COLS = A_COLS + P_COLS  # 8192

BF16 = ml_dtypes.bfloat16

_CACHE = {}

# 96-block -> 128-chunk repacking plan. Blocks 0/4 are chunk-aligned and
# copy straight from PSUM on ACT; the rest are assembled by 32-wide
# partition-shifted DVE copies (src_lo, src_hi, chunk, dst_partition) --
# the DVE's 32-lane bank can write any quadrant, wider copies cannot
# shift across quadrants.
DIRECT = {0: 0, 4: 3}  # block -> chunk (dst partition 0)
ASM32 = {
    1: [(0, 32, 0, 96), (32, 64, 1, 0), (64, 96, 1, 32)],
    2: [(0, 32, 1, 64), (32, 64, 1, 96), (64, 96, 2, 0)],
    3: [(0, 32, 2, 32), (32, 64, 2, 64), (64, 96, 2, 96)],
    5: [(0, 32, 3, 96), (32, 64, 4, 0), (64, 96, 4, 32)],
    6: [(0, 32, 4, 64), (32, 64, 4, 96), (64, 96, 5, 0)],
    7: [(0, 32, 5, 32), (32, 64, 5, 64), (64, 96, 5, 96)],
}


def _build_amat(block_weights, gates):
    """Per-block real [224, 224] spatial-W operator."""
    g = 1.0 / (1.0 + np.exp(-gates.astype(np.float64)))
    F = np.fft.rfft(np.eye(Ww), axis=1, norm="ortho")  # (224, 113)
    A = np.zeros((NB, Ww, Ww), np.float64)
    for b in range(NB):
        T = np.zeros((NMODES, NMODES), np.complex128)
        T[:M, :M] = g[b] * block_weights[b].astype(np.float64)
        for k in range(M, NMODES):
            T[k, k] = 1.0
        A[b] = np.fft.irfft(F @ T, n=Ww, axis=1, norm="ortho")
    return A, g


def _bias_const_rows(block_bias, g):
    """Constant added to spatial rows n_h == 0, per block: (NB, 224)."""
    rows = np.zeros((NB, Ww), np.float64)
    for b in range(NB):
        z = np.zeros(NMODES, np.complex128)
        z[:M] = g[b] * block_bias[b].astype(np.float64) * (1.0 + 1.0j)
        rows[b] = np.sqrt(Hh) * np.fft.irfft(z, n=Ww, norm="ortho")
    return rows


def _pack_weights(A, P):
    """[128, 8192] bf16: A chunks (k=0 rows 0:128, k=1 rows 128:224),
    then P in 6 K=128 chunks in natural channel order."""
    wc = np.zeros((128, WC_COLS), np.float32)
    for k in range(2):
        for b in range(NB):
            blk = A[b, k * 128 : min((k + 1) * 128, Ww), :]  # (128|96, 224)
            wc[: blk.shape[0], k * NB * Ww + b * Ww : k * NB * Ww + (b + 1) * Ww] = blk
    for k in range(NS):
        wc[:, A_COLS + k * C : A_COLS + (k + 1) * C] = P[k * 128 : (k + 1) * 128, :]
    return wc.astype(BF16)


def _elide_redundant_waits(nc):
    """Drop per-instruction semaphore waits already implied by the
    instruction's other waits (transitively, via the wait chains of the
    instructions that perform the increments). Tile's sem assignment is
    per-proc minimal but not transitively minimal across procs, and
    walrus's per-instruction sync-command budget is tiny (matmul fits
    only one wait + one update)."""
    fn = nc.m.functions[0]
    implied = {}  # sem name -> [state dict after k-th increment]
    engine_state = {}  # engine -> folded state of prior instructions' waits

    def state_of(sem, v):
        lst = implied.get(sem)
        if not lst or v <= 0:
            return {}
        return lst[min(v, len(lst)) - 1]

    def fold(dst, src):
        for s, v in src.items():
            if dst.get(s, 0) < v:
                dst[s] = v

    own_updates = {}  # engine -> {sem: count of updates emitted by this engine}
    sem_updaters = {}  # sem -> set of (engine, is_dma) that updated it
    for blk in fn.blocks:
        for inst in blk.instructions:
            si = inst.sync_info
            eng = getattr(inst, "engine", None)
            is_dma = "DMA" in type(inst).__name__
            waits = list(si.on_wait or []) if si else []
            # prune waits on this engine's own completion sem: a compute
            # engine executes serially, so all its prior updates are done
            # by the time this instruction runs. Only valid when every
            # updater of the sem so far was this engine's synchronous
            # (non-DMA) instructions.
            if eng is not None and not is_dma and waits:
                keep0 = []
                for w in waits:
                    ups = sem_updaters.get(w.ant_name)
                    if (
                        w.wait_value is not None
                        and ups is not None
                        and ups == {(eng, False)}
                        and own_updates.get(eng, {}).get(w.ant_name, 0)
                        >= w.wait_value
                    ):
                        continue
                    keep0.append(w)
                if len(keep0) != len(waits):
                    si.on_wait = keep0
                    waits = keep0
            my = dict(engine_state.get(eng, {}))
            for w in waits:
                if w.wait_value is None:
                    continue
                fold(my, {w.ant_name: w.wait_value})
                fold(my, state_of(w.ant_name, w.wait_value))
            if len(waits) > 1 and all(w.wait_value is not None for w in waits):
                keep = []
                for w in waits:
                    others = dict(engine_state.get(eng, {}))
                    for w2 in waits:
                        if w2 is w:
                            continue
                        fold(others, {w2.ant_name: w2.wait_value})
                        fold(others, state_of(w2.ant_name, w2.wait_value))
                    if others.get(w.ant_name, -1) >= w.wait_value:
                        continue
                    keep.append(w)
                if len(keep) != len(waits):
                    si.on_wait = keep
            if eng is not None:
                engine_state[eng] = my
            for u in (si.on_update or []) if si else []:
                nm = u.ant_name
                lst = implied.setdefault(nm, [])
                prev = dict(lst[-1]) if lst else {}
                fold(prev, my)
                n = u.update_value or 1
                prev[nm] = len(lst) + n
                for _ in range(int(n)):
                    lst.append(prev)
                if eng is not None:
                    eu = own_updates.setdefault(eng, {})
                    eu[nm] = eu.get(nm, 0) + int(n)
                sem_updaters.setdefault(nm, set()).add((eng, is_dma))


def _build_nc():
    nc = bass.Bass("TRN2", target_bir_lowering=False)
    x_ext = nc.declare_dram_parameter(
        "x", [ROWS_CORE, Ww, C], mybir.dt.bfloat16, isOutput=False
    )
    w_ext = nc.declare_dram_parameter(
        "wconst", [128, WC_COLS], mybir.dt.bfloat16, isOutput=False
    )
    out_ext = nc.declare_dram_parameter(
        "out", [TOK_CORE, C], mybir.dt.bfloat16, isOutput=True
    )

    def a_sl(k, b):
        return slice(k * NB * Ww + b * Ww, k * NB * Ww + (b + 1) * Ww)

    def p_sl(k, lo, hi):
        return slice(A_COLS + k * C + lo, A_COLS + k * C + hi)

    with tile.TileContext(nc) as tc:
        with (
            tc.tile_pool(name="const", bufs=1) as const_pool,
            tc.tile_pool(name="xin", bufs=2) as x_pool,
            tc.tile_pool(name="xs", bufs=2) as xs_pool,
            tc.tile_pool(name="gps", bufs=2, space="PSUM") as g_psum,
            tc.tile_pool(name="ops", bufs=2, space="PSUM") as o_psum,
            tc.tile_pool(name="osb", bufs=3) as out_pool,
        ):
            wc = const_pool.tile([128, WC_COLS], mybir.dt.bfloat16)
            nc.sync.dma_start(wc[:, :], w_ext[:, :])

            def load_x(g):
                xt = x_pool.tile([128, RG, 2, C], mybir.dt.bfloat16, tag="xin")
                src = x_ext[g * RG : (g + 1) * RG, :, :]
                nc.gpsimd.dma_start(
                    xt[:, :, 0, :], src[:, 0:128, :].rearrange("r p c -> p r c")
                )
                nc.gpsimd.dma_start(
                    xt[0:96, :, 1, :], src[:, 128:224, :].rearrange("r p c -> p r c")
                )
                return xt

            def stage1(g, xt):
                """returns xs [128, 6*TG+1] bf16 (6 channel chunks of 128)"""
                xs = xs_pool.tile([128, NS * TG + 1], mybir.dt.bfloat16, tag="xs")
                nc.vector.tensor_copy(xs[0:1, NS * TG :], wc[0:1, 0:1])
                for b in range(NB):
                    # [96, 2 banks, 512]: rows r at (r//2, (r%2)*224)
                    pg = g_psum.tile([96, 2, 512], mybir.dt.float32, tag="gps")
                    for r in range(RG):
                        o = (r % 2) * Ww
                        for k in range(2):
                            kp = 128 if k == 0 else 96
                            nc.tensor.matmul(
                                pg[:, r // 2, o : o + Ww],
                                lhsT=xt[0:kp, r, k, b * BS : (b + 1) * BS],
                                rhs=wc[0:kp, a_sl(k, b)],
                                start=(k == 0),
                                stop=(k == 1),
                            )
                    if b in DIRECT:
                        s = DIRECT[b]
                        nc.scalar.copy(
                            xs[0:96, s * TG : (s + 1) * TG], pg[:, :, 0 : 2 * Ww]
                        )
                    else:
                        for (lo, hi, s, dp) in ASM32[b]:
                            nc.vector.tensor_copy(
                                xs[dp : dp + hi - lo, s * TG : (s + 1) * TG],
                                pg[lo:hi, :, 0 : 2 * Ww],
                            )
                return xs

            def stage2(g, xs):
                ots = []
                for j in range(TCH):
                    po1 = o_psum.tile([128, 512], mybir.dt.float32, tag="po1")
                    po2 = o_psum.tile([128, 256], mybir.dt.float32, tag="po2")
                    for kc in range(NS):
                        lhsT = xs[0:128, kc * TG + j * 128 : kc * TG + (j + 1) * 128]
                        nc.tensor.matmul(
                            po1[:, :], lhsT=lhsT, rhs=wc[0:128, p_sl(kc, 0, 512)],
                            start=(kc == 0), stop=(kc == NS - 1),
                        )
                        nc.tensor.matmul(
                            po2[:, :], lhsT=lhsT, rhs=wc[0:128, p_sl(kc, 512, 768)],
                            start=(kc == 0), stop=(kc == NS - 1),
                        )
                    # one spare column: the 1-element gate copy absorbs
                    # ot's slot-free (DMA) wait on the ACT queue so the
                    # real copies only carry the PE wait (walrus allows
                    # one wait per Activation), without WAW overlap.
                    ot = out_pool.tile([128, C + 1], mybir.dt.bfloat16, tag="osb")
                    nc.scalar.copy(ot[0:1, C : C + 1], wc[0:1, 0:1])
                    nc.scalar.copy(ot[:, 0:512], po1[:, :])
                    nc.scalar.copy(ot[:, 512:768], po2[:, :])
                    t0 = g * TG + j * 128
                    nc.sync.dma_start(out_ext[t0 : t0 + 128, :], ot[:, 0:C])
                    ots.append(ot)
                return ots

            # software pipeline: stage1(g) then stage2(g-1) in PE program order
            xs_prev = None
            xt = load_x(0)
            for g in range(GROUPS):
                xs_cur = stage1(g, xt)
                if g + 1 < GROUPS:
                    xt = load_x(g + 1)
                if xs_prev is not None:
                    stage2(g - 1, xs_prev)
                xs_prev = xs_cur
            last_ots = stage2(GROUPS - 1, xs_prev)
            # tail joins: tiny ACT writes into the last out tiles make the
            # ACT queue observe the final out-DMA completions (WAR), so the
            # kernel-tail Drain's DMA-lane waits become implied and are
            # elided (walrus allows only one wait on Drain).
            for ot in last_ots[-3:]:
                nc.scalar.copy(ot[0:1, 0:1], wc[0:1, 0:1])

    _elide_redundant_waits(nc)
    return nc


def kernel(x, block_weights, block_bias, gates, proj_w, proj_b, rescale):
    x = np.asarray(x)
    A, g = _build_amat(np.asarray(block_weights), np.asarray(gates))
    P = float(rescale) * np.asarray(proj_w, np.float64).T + np.eye(C)
    w_dev = _pack_weights(A, P)

    # shard 56 image rows per core
    x_rows = np.ascontiguousarray(x.reshape(TOK // Ww, Ww, C).astype(BF16))

    if "nc" not in _CACHE:
        _CACHE["nc"] = _build_nc()
    nc = _CACHE["nc"]

    in_maps = []
    for i in range(N_CORES):
        in_maps.append(
            {
                "x": x_rows[i * ROWS_CORE : (i + 1) * ROWS_CORE],
                "wconst": w_dev,
            }
        )
    res = run_bass_kernel_spmd(
        nc,
        in_maps,
        core_ids=list(range(N_CORES)),
        trace=bool(_CACHE.get("trace", False)),
        **_CACHE.get("trace_kwargs", {}),
    )
    _CACHE["last_results"] = res
    out = np.concatenate([r["out"] for r in res.results], axis=0)
    out = out.reshape(B, Hh * Ww, C).astype(np.float32)

    # host-side constant corrections (zero for the reference inputs)
    bb = np.asarray(block_bias)
    pb = np.asarray(proj_b)
    if np.any(bb) or np.any(pb):
        const = np.zeros((Hh * Ww, C), np.float64)
        if np.any(bb):
            rows = _bias_const_rows(bb, g)  # (NB, 224)
            cr = np.zeros((Ww, C), np.float64)
            for b in range(NB):
                cr[:, b * BS : (b + 1) * BS] = rows[b][:, None]
            # affects tokens with n_h == 0: tokens 0..223 of each batch image
            const[0:Ww, :] = cr @ P  # x_const goes through out = x_const @ P
        add = const[None, :, :] + float(rescale) * pb.astype(np.float64)[None, None, :]
        out = (out.astype(np.float64) + add).astype(np.float32)
    return out

